# revision 23
# baseline (speedup 1.0000x reference)
"""Trainium2 Bass kernel for nn_Disc_edge4 (3-layer edge-conditioned GNN + readout).

Strategy (per core; batch-parallel over 8 cores, 2 batch elements per core):
  - Layout: features on partitions, both batch elements stacked (b0 -> parts
    0..63, b1 -> 64..127).  Edge fields are [128, N*N] with column c = j*128+i.
  - The edge MLP  m @ We  (m = [x_i | x_j | e]) is decomposed as
        e @ We_c  +  x_i @ We_a  +  x_j @ We_b
    and computed as 3 accumulating PE matmuls into PSUM.  The x_i / x_j terms
    use broadcast (step-0) access patterns on xT2, so no [N*N, Fn] field is
    ever materialized.
  - Layer 0 folds the -1e9*(1-adj) mask into the e-matmul as 2 extra
    contraction rows (K=66).  Layers 1/2 have K=128 (full), so the mask is a
    fused DVE scalar_tensor_tensor: e_next = relu(z+be) * mask.
  - ACT evacuates PSUM with fused relu+bias.  DVE does mask-mult and the
    per-target-node aggregation (strided tensor_reduce over j).
  - Node MLP, degree normalization, readout MLP are all tiny [128,128] ops.
  - No collectives: host slices batches per core and concatenates outputs.

kernel(**inputs) takes the FULL unsharded inputs (as in reference.setup_inputs)
and returns the full [16] output.
"""
import os
import numpy as np

import concourse.bass as bass
import concourse.bacc as bacc
import concourse.mybir as mybir
import concourse.tile as tile
from concourse.bass_utils import run_bass_kernel_spmd

N = 128
NN = N * N
CHUNK = int(os.environ.get("K_CHUNK", "1024"))   # columns per psum chunk
NCH = NN // CHUNK       # 16 chunks per layer
NEG = -1e9
NCORES = 8
F32 = mybir.dt.float32
F32R = mybir.dt.float32r
BF16 = mybir.dt.bfloat16
FP8 = mybir.dt.float8e4
AF = mybir.ActivationFunctionType
ALU = mybir.AluOpType

# ---------------------------------------------------------------------------
# Bass program (identical for every core)
# ---------------------------------------------------------------------------

_cache = {}


def build_nc():
    nc = bacc.Bacc(None, target_bir_lowering=False, debug=False)

    # ---- DRAM parameters (per-core) ----
    d_rhs0 = nc.dram_tensor("rhs0", [NCH, 66, CHUNK], F32R, kind="ExternalInput")
    d_mask = nc.dram_tensor("mask2", [NCH, 128, CHUNK], FP8, kind="ExternalInput")
    d_xT0 = nc.dram_tensor("xT2_0", [64, N], F32R, kind="ExternalInput")
    d_rdegb = nc.dram_tensor("rdegb", [128, N], F32, kind="ExternalInput")
    d_biases = nc.dram_tensor("biases", [128, 8], F32, kind="ExternalInput")
    d_b3 = nc.dram_tensor("b3b", [2, 1], F32, kind="ExternalInput")

    wspecs = {
        "w_e0": [66, 128], "w_xi0": [64, 128], "w_xj0": [64, 128],
        "w_e1": [128, 128], "w_xi1": [128, 128], "w_xj1": [128, 128],
        "w_e2": [128, 128], "w_xi2": [128, 128], "w_xj2": [128, 128],
        "w_nx0": [64, 128], "w_na0": [128, 128],
        "w_nx1": [128, 128], "w_na1": [128, 128],
        "w_r1": [128, 128], "w_r2": [128, 128], "w_r3": [128, 2],
    }
    d_w = {k: nc.dram_tensor(k, sh, F32 if k.startswith("w_r") else F32R,
                             kind="ExternalInput")
           for k, sh in wspecs.items()}

    d_out = nc.dram_tensor("out", [2, 1], F32, kind="ExternalOutput")

    from contextlib import ExitStack
    with tile.TileContext(nc) as tc, ExitStack() as ctx:
        const = ctx.enter_context(tc.tile_pool(name="const", bufs=1))
        big = ctx.enter_context(tc.tile_pool(name="big", bufs=1))
        tpool = ctx.enter_context(tc.tile_pool(name="tp", bufs=(3 if CHUNK == 1024 else 2)))
        small = ctx.enter_context(tc.tile_pool(name="small", bufs=1))
        zpsum = ctx.enter_context(tc.tile_pool(name="zpsum", bufs=(4 if CHUNK == 1024 else 2), space="PSUM"))
        spsum = None

        # ---- load constants ----
        # L0-critical constants go first on the SP queue (ahead of rhs0
        # chunks); everything else rides the otherwise-idle gpsimd queue.
        L0_CRIT = ("w_e0", "w_xi0", "w_xj0")
        w = {}
        for k, sh in wspecs.items():
            w[k] = const.tile(sh, F32 if k.startswith("w_r") else F32R,
                              tag=k, name=k)
        xT0 = const.tile([64, N], F32R, tag="xT0")
        biases = const.tile([128, 8], F32, tag="biases")
        rdegb = const.tile([128, N], F32, tag="rdegb")
        b3sb = const.tile([2, 1], F32, tag="b3sb")
        for k in L0_CRIT:
            nc.sync.dma_start(w[k][:], d_w[k][:])
        nc.sync.dma_start(xT0[:], d_xT0[:])
        nc.sync.dma_start(biases[:], d_biases[:])
        for k in wspecs:
            if k not in L0_CRIT:
                nc.gpsimd.dma_start(w[k][:], d_w[k][:])
        nc.gpsimd.dma_start(rdegb[:], d_rdegb[:])
        nc.gpsimd.dma_start(b3sb[:], d_b3[:])

        # ---- big SBUF tensors ----
        # rhs0_buf doubles as e_b (layer-1 output) once layer 0 has consumed it
        rhs0_buf = big.tile([128, NN], F32R, tag="rhs0")
        e_a = big.tile([128, NN], F32R, tag="e_a")
        mask_sb = big.tile([128, NN], FP8, tag="mask")

        for k in range(NCH):
            sl = slice(k * CHUNK, (k + 1) * CHUNK)
            nc.sync.dma_start(rhs0_buf[0:66, sl], d_rhs0[k])
            nc.sync.dma_start(mask_sb[:, sl], d_mask[k])

        def edge_layer(lidx, e_in, k_in, w_e, w_xi, w_xj, xT2, be_col, e_out):
            """One edge-update layer over the full [*, NN] field.

            e_in:  SBUF tile whose first k_in partitions hold the matmul rhs
            e_out: SBUF [128, NN] tile for e_next, or None (layer 2) to keep
                   only the reduction.
            Returns nothing; writes row-block partial sums into `partials`.
            """
            kxi = w_xi.shape[0]
            acc = small.tile([128, N if e_out is not None else (NCH + 1)], F32,
                             tag=f"acc{lidx}_{_rep}", name=f"acc{lidx}")
            # last chunk split into 512-halves to shorten the drain chain
            pieces = [(k * CHUNK, CHUNK) for k in range(NCH - 1)]
            pieces += [((NCH - 1) * CHUNK, 512), ((NCH - 1) * CHUNK + 512, 512)]
            for pi, (p0, plen) in enumerate(pieces):
                z = zpsum.tile([128, plen], F32, tag="z", name="z")
                for g in range(plen // 512):
                    c0 = p0 + g * 512
                    jb0 = c0 // N          # first j-block of this group
                    zg = z[:, g * 512:(g + 1) * 512]
                    rhs_e = e_in[0:k_in, c0:c0 + 512]
                    nc.tensor.matmul(zg, w_e[:], rhs_e, start=True, stop=False)
                    # x_i varies with i inside each 128-block -> broadcast j
                    rhs_xi = xT2[:, :].unsqueeze(1).broadcast_to([kxi, 4, N])
                    nc.tensor.matmul(zg.rearrange("p (a b) -> p a b", a=4),
                                     w_xi[:], rhs_xi, start=False, stop=False)
                    # x_j constant inside each 128-block -> broadcast i
                    rhs_xj = xT2[:, jb0:jb0 + 4].unsqueeze(2).broadcast_to([kxi, 4, N])
                    nc.tensor.matmul(zg.rearrange("p (a b) -> p a b", a=4),
                                     w_xj[:], rhs_xj, start=False, stop=True)
                sl = slice(p0, p0 + plen)
                if lidx == 0:
                    # mask already folded in via -1e9 rows: relu finishes it
                    nc.scalar.activation(e_out[:, sl], z[:], AF.Relu, bias=be_col)
                    red_src = e_out[:, sl]
                else:
                    t = tpool.tile([128, plen], F32, tag="t", name="t")
                    nc.scalar.activation(t[:], z[:], AF.Relu, bias=be_col)
                    if e_out is not None:
                        dst = e_out[:, sl]
                        nc.gpsimd.tensor_tensor(
                            dst, t[:], mask_sb[:, sl], op=ALU.mult)
                        red_src = dst
                    else:
                        # layer 2 needs only the global edge sum: DVE stt
                        # (HW-validated opcode) fuses mask-mult + chunk sum
                        t2 = tpool.tile([128, plen], F32, tag="t2", name="t2")
                        nc.vector.scalar_tensor_tensor(
                            t2[:], t[:], 0.0, mask_sb[:, sl],
                            op0=ALU.bypass, op1=ALU.mult,
                            accum_out=acc[:, pi:pi + 1])
                        continue
                # partial row sums over the j-blocks in this chunk
                part = tpool.tile([128, N], F32, tag="part", name="part")
                nc.vector.tensor_reduce(
                    part[:],
                    red_src.rearrange("p (jb i) -> p i jb", i=N),
                    axis=mybir.AxisListType.X, op=ALU.add)
                if pi == 0:
                    nc.vector.tensor_scalar(acc[:], part[:], 1.0, None, op0=ALU.mult)
                else:
                    nc.vector.tensor_tensor(acc[:], acc[:], part[:], op=ALU.add)
            return acc

        def node_mlp(w_nx, w_na, xT2, bn_col, out_x, rowsums):
            agg = small.tile([128, N], F32R, tag="agg")
            nc.vector.tensor_tensor(agg[:], rowsums[:], rdegb[:], op=ALU.mult)
            zp = zpsum.tile([128, N], F32, tag="z", name="zp")
            nc.tensor.matmul(zp[:], w_nx[:], xT2[:, :], start=True, stop=False)
            nc.tensor.matmul(zp[:], w_na[:], agg[:], start=False, stop=True)
            nc.scalar.activation(out_x[:], zp[:], AF.Relu, bias=bn_col)

        NLAYERS = int(os.environ.get("K_LAYERS", "3"))
        # ---- layer 0 ----
        edge_layer(0, rhs0_buf, 66, w["w_e0"], w["w_xi0"], w["w_xj0"],
                   xT0, biases[:, 0:1], e_a)
        xT1 = small.tile([128, N], F32R, tag="xT1")
        node_mlp(w["w_nx0"], w["w_na0"], xT0, biases[:, 3:4], xT1)

        if NLAYERS >= 2:
            # ---- layer 1 (writes into rhs0_buf, now dead) ----
            edge_layer(1, e_a, 128, w["w_e1"], w["w_xi1"], w["w_xj1"],
                       xT1, biases[:, 1:2], rhs0_buf)
            xT2_ = small.tile([128, N], F32R, tag="xT2_")
            node_mlp(w["w_nx1"], w["w_na1"], xT1, biases[:, 4:5], xT2_)

        if NLAYERS >= 3:
            # ---- layer 2 (no stored e3; reductions only) ----
            edge_layer(2, rhs0_buf, 128, w["w_e2"], w["w_xi2"], w["w_xj2"],
                       xT2_, biases[:, 2:3], None)

        # ---- readout ----
        g = small.tile([128, 1], F32, tag="g")
        nc.vector.tensor_reduce(g[:], partials[:, :],
                                axis=mybir.AxisListType.X, op=ALU.add)
        # 1/16384 is folded into w_r1 host-side
        pool_ = spsum if spsum is not None else zpsum
        h1p = pool_.tile([128, 1], F32, tag="sp" if spsum is not None else "z", name="h1p")
        nc.tensor.matmul(h1p[:], w["w_r1"][:], g[:], start=True, stop=True)
        h1 = small.tile([128, 1], F32, tag="h1")
        nc.vector.tensor_scalar(h1[:], h1p[:], biases[:, 5:6], 0.0,
                                op0=ALU.add, op1=ALU.max)
        h2p = pool_.tile([128, 1], F32, tag="sp" if spsum is not None else "z", name="h2p")
        nc.tensor.matmul(h2p[:], w["w_r2"][:], h1[:], start=True, stop=True)
        h2 = small.tile([128, 1], F32, tag="h2")
        nc.vector.tensor_scalar(h2[:], h2p[:], biases[:, 6:7], 0.0,
                                op0=ALU.add, op1=ALU.max)
        op = pool_.tile([2, 1], F32, tag="sp" if spsum is not None else "z", name="op")
        nc.tensor.matmul(op[:], w["w_r3"][:], h2[:], start=True, stop=True)
        osb = small.tile([2, 1], F32, tag="osb")
        nc.vector.tensor_tensor(osb[:], op[:], b3sb[:], op=ALU.add)
        nc.sync.dma_start(d_out[:], osb[:])

    nc.compile()
    return nc


# ---------------------------------------------------------------------------
# Host-side prep
# ---------------------------------------------------------------------------

def _blkdiag(A):
    K, M = A.shape
    out = np.zeros((2 * K, 2 * M), dtype=np.float32)
    out[:K, :M] = A
    out[K:, M:] = A
    return out


def _prep_core(edge_index, x, edge_attr, W):
    """Build the per-core input map. edge_index [2,N,N] i32, x [2,N,32],
    edge_attr [2,N,N,32]."""
    import ml_dtypes
    adj = edge_index.astype(np.float32)
    m = {}
    eT0 = np.transpose(edge_attr, (0, 3, 2, 1)).reshape(64, NN)
    adjbar = (1.0 - np.transpose(adj, (0, 2, 1))).reshape(2, NN)
    rhs0_flat = np.concatenate([eT0, adjbar], 0)           # [66, NN]
    m["rhs0"] = np.ascontiguousarray(
        rhs0_flat.reshape(66, NCH, CHUNK).transpose(1, 0, 2))
    mask01 = np.transpose(adj, (0, 2, 1)).reshape(2, NN)
    mask_flat = np.repeat(mask01, 64, axis=0).astype(ml_dtypes.float8_e4m3)
    m["mask2"] = np.ascontiguousarray(
        mask_flat.reshape(128, NCH, CHUNK).transpose(1, 0, 2))
    m["xT2_0"] = np.ascontiguousarray(
        np.concatenate([x[0].T, x[1].T], 0)).astype(np.float32)
    rdeg = 1.0 / np.maximum(adj.sum(-1), 1.0)          # [2, N]
    m["rdegb"] = np.ascontiguousarray(np.repeat(rdeg, 64, axis=0)).astype(np.float32)

    (We0, be0, Wn0, bn0, We1, be1, Wn1, bn1, We2, be2,
     W1, b1, W2, b2, W3, b3) = W
    def esplit(We, Fn):
        return We[:Fn], We[Fn:2 * Fn], We[2 * Fn:]
    Wea0, Web0, Wc0 = esplit(We0, 32)
    Wea1, Web1, Wc1 = esplit(We1, 64)
    Wea2, Web2, Wc2 = esplit(We2, 64)

    le0 = np.zeros((66, 128), np.float32)
    le0[:64] = _blkdiag(Wc0)
    le0[64, :64] = NEG
    le0[65, 64:] = NEG
    m["w_e0"] = le0
    m["w_xi0"] = _blkdiag(Wea0)
    m["w_xj0"] = _blkdiag(Web0)
    m["w_e1"] = _blkdiag(Wc1)
    m["w_xi1"] = _blkdiag(Wea1)
    m["w_xj1"] = _blkdiag(Web1)
    m["w_e2"] = _blkdiag(Wc2)
    m["w_xi2"] = _blkdiag(Wea2)
    m["w_xj2"] = _blkdiag(Web2)
    m["w_nx0"] = _blkdiag(Wn0[:32])
    m["w_na0"] = _blkdiag(Wn0[32:])
    m["w_nx1"] = _blkdiag(Wn1[:64])
    m["w_na1"] = _blkdiag(Wn1[64:])
    m["w_r1"] = _blkdiag(W1) / NN
    m["w_r2"] = _blkdiag(W2)
    m["w_r3"] = _blkdiag(W3)
    biases = np.zeros((128, 8), np.float32)
    for col, b in enumerate([be0, be1, be2, bn0, bn1, b1, b2]):
        biases[:, col] = np.concatenate([b, b])
    m["biases"] = biases
    m["b3b"] = np.concatenate([b3, b3]).reshape(2, 1).astype(np.float32)
    return m


def make_in_maps(edge_index, x, edge_attr, W):
    return [_prep_core(np.asarray(edge_index[2 * c:2 * c + 2]),
                       np.asarray(x[2 * c:2 * c + 2]),
                       np.asarray(edge_attr[2 * c:2 * c + 2]), W)
            for c in range(NCORES)]


def kernel(edge_index, x, edge_attr,
           We0, be0, Wn0, bn0, We1, be1, Wn1, bn1, We2, be2, Wn2, bn2,
           W1, b1, W2, b2, W3, b3, **run_kwargs):
    W = tuple(np.asarray(a, np.float32) for a in
              (We0, be0, Wn0, bn0, We1, be1, Wn1, bn1, We2, be2,
               W1, b1, W2, b2, W3, b3))
    in_maps = make_in_maps(np.asarray(edge_index), np.asarray(x),
                           np.asarray(edge_attr), W)
    if "nc" not in _cache:
        _cache["nc"] = build_nc()
    nc = _cache["nc"]
    _cache["last_in_maps"] = in_maps
    res = run_bass_kernel_spmd(nc, in_maps, list(range(NCORES)), **run_kwargs)
    _cache["last_result"] = res
    out = np.concatenate([np.asarray(r["out"]).reshape(2) for r in res.results])
    return out.astype(np.float32)
